# revision 45
# baseline (speedup 1.0000x reference)
"""Multi-head attention with RoPE (B=4, N=2048, C=1024, H=16, d=64) on 8
Trainium2 NeuronCores.

Sharding: tensor-parallel over heads — each core computes 2 of the 16 heads
(Wq/Wkv sharded column-wise, Wout row-wise). Each core returns a partial
yT = (out_h @ Wout_h).T (bf16) over the full batch; the host sums the 8
partials in fp32.

Per-core kernel: a single software-pipelined schedule built around keeping
the Activation engine (softmax exp — the per-core throughput limiter at
~1.33us per 128x1024 tile) 100% fed while the PE stays dense enough to hold
its fast p-state:

  unit (b, qc, kt) = one S^T tile: 2 heads x 128 keys x 512 queries.
  Per unit the emitter issues, in order:
    QK(u+2)   [PE]  k_tile.T@q packed 2-heads via tile_position row groups
    exp(u+1)  [ACT] PSUM->SBUF bf16, scale=1/8 folded; no max-subtraction
                    needed (|S|*scale < 3)
    PV(u)     [PE]  O^T + fused denominators (ones-column trick) accumulate
    + ~1 outproj matmul of the previous query chunk (spread, not burst)
    + ~1-2 projection matmuls of the NEXT batch (x@W chunks, RoPE on tails)
  so the three matmul streams interleave at instruction granularity and the
  2-deep S-tile PSUM pipeline never lets ACT starve.

  PSUM (8 banks): S tiles 2x2, proj accum 1, O-pair 2, outproj 1.
  ACT runs exp ONLY — all PSUM evacuations (RoPE raw, vT, yT) are on
  DVE; rotate-half via partition-swapped SBUF DMA copies (sign folded into
  the host-prepared sin table); v transposed token-major via a DRAM bounce
  + XBAR DMA transpose with interleaved ones columns so the PV matmul
  (M=65) also produces the softmax denominators.
"""

import numpy as np
import ml_dtypes
from collections import deque
from contextlib import ExitStack

import concourse.bass as bass
import concourse.tile as tile
from concourse import bacc, mybir
from concourse.bass_utils import run_bass_kernel_spmd

P = 128
B, NSEQ, C = 4, 2048, 1024
H, D = 16, 64
NTOK = B * NSEQ
KO = C // P       # 8 contraction chunks of x@W
QC = 512          # query-chunk width
NKT = NSEQ // P   # 16 key tiles per chunk
NQC = NSEQ // QC  # 4 query chunks per batch
FC = C // P       # 8 output-feature chunks
VW = 160          # vtok row width: [v_h0 | 1 | v_h1 | 1 | pad]
BF = mybir.dt.bfloat16
F32 = mybir.dt.float32
NB = B
NU = NQC * NKT    # 64 units per batch
NTOT = NB * NU
LOOK = 2          # QK lookahead (bounded by the 2 S-tile PSUM slots)
FILL_DEN = 60     # fills drain over this many units of the batch


def _build():
    nc = bacc.Bacc("TRN2", target_bir_lowering=False, debug=False)

    xT = nc.dram_tensor("xT", [C, NTOK], BF, kind="ExternalInput").ap()
    wq = nc.dram_tensor("wq", [C, P], BF, kind="ExternalInput").ap()
    wk = nc.dram_tensor("wk", [C, P], BF, kind="ExternalInput").ap()
    wv = nc.dram_tensor("wv", [C, P], BF, kind="ExternalInput").ap()
    wout = nc.dram_tensor("wout", [P, C], BF, kind="ExternalInput").ap()
    cos2 = nc.dram_tensor("cos2", [P, NSEQ], F32, kind="ExternalInput").ap()
    sin2s = nc.dram_tensor("sin2s", [P, NSEQ], F32, kind="ExternalInput").ap()
    yT = nc.dram_tensor("yT", [C, NTOK], BF, kind="ExternalOutput").ap()

    with ExitStack() as ctx:
        tc = ctx.enter_context(tile.TileContext(nc))
        consts = ctx.enter_context(tc.tile_pool(name="consts", bufs=1))
        xpool = ctx.enter_context(tc.tile_pool(name="xpool", bufs=2))
        qkpool = ctx.enter_context(tc.tile_pool(name="qkpool", bufs=2))
        vpool = ctx.enter_context(tc.tile_pool(name="vpool", bufs=2))
        rope = ctx.enter_context(tc.tile_pool(name="rope", bufs=2))
        pexp_pool = ctx.enter_context(tc.tile_pool(name="pexp", bufs=9))
        onorm_pool = ctx.enter_context(tc.tile_pool(name="onorm", bufs=2))
        yt_pool = ctx.enter_context(tc.tile_pool(name="yt", bufs=3))
        small = ctx.enter_context(tc.tile_pool(name="small", bufs=2))
        dram = ctx.enter_context(tc.tile_pool(name="dram", bufs=2, space="DRAM"))
        ps = ctx.enter_context(tc.tile_pool(name="ps", bufs=1, space="PSUM"))

        # ---- constants ----
        wq_sb = consts.tile([P, KO, P], BF, tag="wq")
        wk_sb = consts.tile([P, KO, P], BF, tag="wk")
        wv_sb = consts.tile([P, KO, P], BF, tag="wv")
        wout_sb = consts.tile([P, FC, P], BF, tag="wout")
        cos_sb = consts.tile([P, NSEQ], F32, tag="cos")
        sin_sb = consts.tile([P, NSEQ], F32, tag="sin")
        nc.sync.dma_start(wq_sb[:], wq.rearrange("(ko p) f -> p ko f", p=P))
        nc.sync.dma_start(wk_sb[:], wk.rearrange("(ko p) f -> p ko f", p=P))
        nc.sync.dma_start(wv_sb[:], wv.rearrange("(ko p) f -> p ko f", p=P))
        nc.sync.dma_start(wout_sb[:], wout.rearrange("r (fc f) -> r fc f", f=P))
        nc.sync.dma_start(cos_sb[:], cos2)
        nc.sync.dma_start(sin_sb[:], sin2s)
        w_sbs = [wq_sb, wk_sb, wv_sb]

        ones_row = consts.tile([1, NSEQ], BF, tag="ones_row")
        nc.vector.memset(ones_row[:], 1.0)
        ones_blk = consts.tile([32, NSEQ], BF, tag="ones_blk")
        nc.vector.memset(ones_blk[:], 1.0)
        vbounces = []
        for i in range(2):
            vb = dram.tile([VW, NSEQ], BF, tag="vbounce", name=f"vb{i}")
            # constant rows written once: the two denominator ones-rows and
            # the 32-multiple pad region
            nc.sync.dma_start(vb[D : D + 1, :], ones_row[:])
            nc.sync.dma_start(vb[2 * D + 1 : 2 * D + 2, :], ones_row[:])
            nc.sync.dma_start(vb[2 * D + 2 : VW, :], ones_blk[: VW - 2 * D - 2, :])
            vbounces.append(vb)

        states = {}
        pss_box = {}
        pexp_box = {}
        po_box = {}
        outq = deque()
        ytq = deque()

        def unit(u):
            return u // NU, (u % NU) // NKT, u % NKT

        def make_state(b):
            xb = xpool.tile([P, KO, NSEQ], BF, tag="xb", name=f"xb{b}")
            qT = qkpool.tile([P, NSEQ], BF, tag="qT", name=f"qT{b}")
            kT = qkpool.tile([P, NSEQ], BF, tag="kT", name=f"kT{b}")
            vT = qkpool.tile([P, NSEQ], BF, tag="vT", name=f"vT{b}")
            vtok = vpool.tile([P, NKT, VW], BF, tag="vtok", name=f"vtok{b}")
            return dict(b=b, xb=xb, qT=qT, kT=kT, vT=vT, vtok=vtok)

        def emit_qk(u):
            b, qc, kt = unit(u)
            st = states[b]
            ensure((b, "k", kt // 8))
            ensure((b, "q", qc // 2))
            qsl = slice(qc * QC, (qc + 1) * QC)
            ksl = slice(kt * P, (kt + 1) * P)
            pss = ps.tile([P, 2, QC], F32, tag="pss", bufs=2, name=f"pss{u}")
            nc.tensor.matmul(
                pss[:, 0, :], st["kT"][0:D, ksl], st["qT"][0:D, qsl],
                start=True, stop=True, tile_position=(0, 0), skip_group_check=True,
            )
            nc.tensor.matmul(
                pss[:, 1, :], st["kT"][D : 2 * D, ksl], st["qT"][D : 2 * D, qsl],
                start=True, stop=True, tile_position=(64, 0), skip_group_check=True,
            )
            pss_box[u] = pss

        def emit_exp(u):
            pss = pss_box.pop(u)
            pexp = pexp_pool.tile([P, 2, QC], BF, tag="pexp", name=f"pexp{u}")
            nc.scalar.activation(
                pexp[:], pss[:], mybir.ActivationFunctionType.Exp, scale=0.125
            )
            pexp_box[u] = pexp

        def emit_norm(b, qc):
            st = states[b]
            po = po_box.pop((b, qc))
            # One copy frees the po PSUM bank ~2.5us after the last PV; the
            # rest of the chain runs from SBUF and can lag without blocking
            # the next chunk's PV accumulation.
            osb = onorm_pool.tile([D + 1, 2 * QC], F32, tag="osb", name=f"osb{b}_{qc}")
            onorm = onorm_pool.tile([P, QC], BF, tag="onorm", name=f"on{b}_{qc}")
            r0 = small.tile([1, QC], F32, tag="r0", name="r0")
            r1 = small.tile([1, QC], F32, tag="r1", name="r1")
            bc0 = small.tile([D, QC], F32, tag="bc0", name="bc0")
            bc1 = small.tile([D, QC], F32, tag="bc1", name="bc1")
            rs = small.tile([1, QC], F32, tag="rs", name="rs")
            rs2 = small.tile([1, QC], F32, tag="rs2", name="rs2")
            # per-head evacuation so head-0's recip/broadcast/mul chain
            # overlaps head-1's copy (reciprocal_approx_fast silently
            # corrupts on sliced inputs — stage the denominator row first)
            nc.vector.tensor_copy(osb[:, 0:QC], po[:, 0, :])
            nc.vector.tensor_copy(rs[:], osb[D : D + 1, 0:QC])
            nc.vector.reciprocal_approx_fast(r0[:], rs[:])
            nc.gpsimd.partition_broadcast(bc0[:], r0[:])
            nc.vector.tensor_copy(osb[:, QC : 2 * QC], po[:, 1, :])
            nc.vector.tensor_copy(rs2[:], osb[D : D + 1, QC : 2 * QC])
            nc.vector.reciprocal_approx_fast(r1[:], rs2[:])
            nc.gpsimd.partition_broadcast(bc1[:], r1[:])
            nc.vector.tensor_mul(onorm[0:D, :], osb[0:D, 0:QC], bc0[:])
            nc.vector.tensor_mul(onorm[D : 2 * D, :], osb[0:D, QC : 2 * QC], bc1[:])
            t0 = b * NSEQ
            last = b == NB - 1 and qc == NQC - 1
            for fc in range(FC):
                def op(fc=fc, onorm=onorm, t0=t0, qc=qc, last=last):
                    if last:
                        # attention is done — reuse the freed S-tile slots so
                        # the tail outproj ping-pongs across 2 banks
                        py = ps.tile([P, QC], F32, tag="pss", bufs=2, name="py")
                    else:
                        py = ps.tile([P, QC], F32, tag="py", bufs=1, name="py")
                    nc.tensor.matmul(
                        py[:], wout_sb[:, fc, :], onorm[:],
                        start=True, stop=True, skip_group_check=True,
                    )
                    yt = yt_pool.tile([P, QC], BF, tag="yt", name="yt")
                    nc.vector.tensor_copy(yt[:], py[:])

                    def dma(fc=fc, yt=yt):
                        nc.sync.dma_start(
                            yT[fc * P : (fc + 1) * P,
                               t0 + qc * QC : t0 + (qc + 1) * QC],
                            yt[:],
                        )
                    ytq.append(dma)
                outq.append(op)

        def emit_pv(u):
            b, qc, kt = unit(u)
            st = states[b]
            ensure((b, "vt", 0))
            if kt == 0:
                po = ps.tile([D + 1, 2, QC], F32, tag="po", bufs=1, name=f"po_{u}")
                po_box[(b, qc)] = po
            po = po_box[(b, qc)]
            pexp = pexp_box.pop(u)
            vtok = st["vtok"]
            nc.tensor.matmul(
                po[:, 0, :], vtok[:, kt, 0 : D + 1], pexp[:, 0, :],
                start=(kt == 0), stop=(kt == NKT - 1), skip_group_check=True,
            )
            nc.tensor.matmul(
                po[:, 1, :], vtok[:, kt, D + 1 : 2 * D + 2], pexp[:, 1, :],
                start=(kt == 0), stop=(kt == NKT - 1), skip_group_check=True,
            )
            if kt == NKT - 1:
                emit_norm(b, qc)

        # ---- fill ops: x loads + projections (+RoPE) of a batch ----
        def chunk_ops(st, f, t4, pair_box=None):
            """One x@W chunk: 8 accumulating matmuls into 1 PSUM bank, split
            into 4 fill-ops of 2, plus an evacuation tail (RoPE for q/k).
            RoPE swap-adds are batched per t4-PAIR: one 1024-wide gpsimd
            DMA-accumulate set per two chunks (halves the SWDGE count)."""
            box = {}
            tsl = slice(t4 * QC, (t4 + 1) * QC)
            ops = []
            for kp in range(0, KO, 2):
                def mmop(kp=kp):
                    if kp == 0:
                        box["pp"] = ps.tile([P, QC], F32, tag="pp", bufs=1, name="pp")
                    pp = box["pp"]
                    for ko in (kp, kp + 1):
                        nc.tensor.matmul(
                            pp[:], w_sbs[f][:, ko, :], st["xb"][:, ko, tsl],
                            start=(ko == 0), stop=(ko == KO - 1),
                            skip_group_check=True,
                        )
                ops.append(("mm", mmop, None))

            if f < 2:
                half = t4 % 2
                def tail1a(half=half):
                    pp = box.pop("pp")
                    dst = st["qT"] if f == 0 else st["kT"]
                    raw = rope.tile([P, QC], F32, tag="raw", name="raw")
                    if half == 0:
                        pair_box["qsp"] = rope.tile(
                            [P, 2 * QC], BF, tag="qsp", name="qsp"
                        )
                    qsp = pair_box["qsp"]
                    nc.vector.tensor_copy(raw[:], pp[:])
                    nc.vector.tensor_mul(dst[:, tsl], raw[:], cos_sb[:, tsl])
                    nc.vector.tensor_mul(
                        qsp[:, half * QC : (half + 1) * QC], raw[:], sin_sb[:, tsl]
                    )
                ops.append(("mm", tail1a, None))
                if half == 1:
                    def tail1b():
                        qsp = pair_box.pop("qsp")
                        dst = st["qT"] if f == 0 else st["kT"]
                        ptsl = slice((t4 - 1) * QC, (t4 + 1) * QC)
                        for blk in range(4):
                            src = (blk ^ 1) * 32
                            nc.gpsimd.dma_start(
                                dst[blk * 32 : blk * 32 + 32, ptsl],
                                qsp[src : src + 32, :],
                                accum_op=mybir.AluOpType.add,
                            )
                    ops.append(
                        ("late", tail1b,
                         (st["b"], "q" if f == 0 else "k", t4 // 2))
                    )
            else:
                def tailv():
                    pp = box.pop("pp")
                    nc.vector.tensor_copy(st["vT"][:, tsl], pp[:])
                ops.append(("mm", tailv, None))
            return ops

        def qk_chunk_pair(st, f, pair):
            pair_box = {}
            ops = chunk_ops(st, f, 2 * pair, pair_box)
            ops += chunk_ops(st, f, 2 * pair + 1, pair_box)
            return ops

        def build_fills(b, head=False):
            st = make_state(b)
            states[b] = st
            t0b = b * NSEQ
            ops = []
            def mkld(t4):
                def ld():
                    w0 = t0b + t4 * QC
                    src = xT[:, w0 : w0 + QC].rearrange("(ko p) t -> p ko t", p=P)
                    nc.sync.dma_start(st["xb"][:, :, t4 * QC : (t4 + 1) * QC], src)
                return ("dma", ld, None)

            def vchunks():
                vops = []
                for t4 in range(NQC):
                    vops.extend(chunk_ops(st, 2, t4))

                def vtrans():
                    vb = vbounces[b % 2]
                    nc.sync.dma_start(vb[0:D, :], st["vT"][0:D, :])
                    nc.sync.dma_start(vb[D + 1 : 2 * D + 1, :], st["vT"][D : 2 * D, :])
                    nc.sync.dma_start_transpose(st["vtok"][:, :, :], vb[:, :])
                vops.append(("dma", vtrans, (b, "vt", 0)))
                return vops

            if head:
                # batch 0: QK(qc0, kt 0-7) needs only k(pair 0) + q(pair 0);
                # the rest drains during the first attention units (the
                # ensure() guards in emit_qk/emit_pv force emission order)
                ops.append(mkld(0))
                ops.append(mkld(1))
                ops.extend(qk_chunk_pair(st, 1, 0))
                ops.extend(qk_chunk_pair(st, 0, 0))
                ops.append(mkld(2))
                ops.append(mkld(3))
                ops.extend(vchunks())
                ops.extend(qk_chunk_pair(st, 1, 1))
                ops.extend(qk_chunk_pair(st, 0, 1))
            else:
                for t4 in range(NQC):
                    ops.append(mkld(t4))
                ops.extend(vchunks())
                for f in (1, 0):
                    for pair in range(NQC // 2):
                        ops.extend(qk_chunk_pair(st, f, pair))
            return deque(ops)

        emitted_keys = set()

        def pop_fill():
            kind, fn, key = fills.popleft()
            fn()
            if key is not None:
                emitted_keys.add(key)
            return kind

        def ensure(key):
            while key not in emitted_keys:
                pop_fill()

        # ---- prime: emit_qk's guards pull in the x loads, k(pair 0) and
        # q(pair 0); everything else drains during the first attention units ----
        fills = build_fills(0, head=True)
        fills_total = len(fills)

        emit_qk(0)
        emit_exp(0)
        emit_qk(1)

        # PV(qc, kt) is emitted at unit-offset PVSLOT[kt] within its own qc —
        # compressed into offsets 6..15 (pexp tiles buffer the lag) so the po
        # accumulator bank of qc is released ~8us before PV(qc+1, 0) reclaims
        # it, hiding the norm-chain latency entirely.
        PVSLOT = [6, 7, 8, 8, 9, 9, 10, 10, 11, 11, 12, 12, 13, 14, 15, 15]
        # first chunk of batch 0: push PVs later so the v-projection +
        # transpose fills can drain before the first PV needs vtok
        PVSLOT0 = [10, 10, 11, 11, 12, 12, 12, 13, 13, 13, 14, 14, 14, 15, 15, 15]
        pv_sched = {}
        for uu in range(NTOT):
            bb, qqc, kkt = unit(uu)
            slot = PVSLOT0 if uu < NKT else PVSLOT
            g = (uu // NKT) * NKT + slot[kkt]
            pv_sched.setdefault(g, []).append(uu)

        # ---- steady-state emission ----
        for u in range(NTOT):
            b, qc, kt = unit(u)
            if u + LOOK < NTOT:
                emit_qk(u + LOOK)
            if u + 1 < NTOT:
                emit_exp(u + 1)
            pvs = pv_sched.get(u, [])
            for uu in pvs:
                emit_pv(uu)
            if kt == 0 and qc == 0 and b + 1 < NB:
                fills.extend(build_fills(b + 1))
                fills_total = len(fills)
            # outproj matmuls consume at offsets 5..12 so the py matmul never
            # reaches the head of the in-order PE queue before onorm is ready;
            # their result DMAs trail 2 units so the SP queue never waits
            if outq and 6 <= kt <= 13:
                outq.popleft()()
            if ytq and (kt >= 7 or kt < 3):
                ytq.popleft()()
            if fills and kt <= 12:
                ub = u % NU
                rem_target = max(0, fills_total - (fills_total * (ub + 1)) // FILL_DEN)
                cap = 3 if kt <= 4 else 2
                popped = 0
                late_done = False
                while fills and len(fills) > rem_target and popped < cap:
                    if fills[0][0] == "late":
                        if late_done or popped >= 2:
                            break  # stagger dependent ops across units
                        late_done = True
                    pop_fill()
                    popped += 1

        while outq:
            outq.popleft()()
        while ytq:
            ytq.popleft()()

    nc.compile()
    return nc


def _host_inputs(x, cos, sin, Wq, Wkv, Wout):
    bf = ml_dtypes.bfloat16
    xT = np.ascontiguousarray(x.reshape(NTOK, C).T).astype(bf)
    cosT = cos.reshape(NSEQ, D).T.astype(np.float32)
    sinT = sin.reshape(NSEQ, D).T.astype(np.float32)
    sign = np.where(np.arange(D)[:, None] < D // 2, -1.0, 1.0).astype(np.float32)
    cos2 = np.ascontiguousarray(np.concatenate([cosT, cosT], 0))
    # sign-folded sin, pre-swapped by the rotate-half permutation so the
    # kernel's DMA-accumulate lands raw[p^32]*sin'[p] at row p
    sin_fold = np.concatenate([sinT * sign, sinT * sign], 0)
    sin2s = np.ascontiguousarray(sin_fold[np.arange(2 * D) ^ (D // 2)])
    maps = []
    for core in range(8):
        c0 = core * P
        maps.append(
            {
                "xT": xT,
                "wq": np.ascontiguousarray(Wq[:, c0 : c0 + P]).astype(bf),
                "wk": np.ascontiguousarray(Wkv[:, c0 : c0 + P]).astype(bf),
                "wv": np.ascontiguousarray(Wkv[:, C + c0 : C + c0 + P]).astype(bf),
                "wout": np.ascontiguousarray(Wout[c0 : c0 + P, :]).astype(bf),
                "cos2": cos2,
                "sin2s": sin2s,
            }
        )
    return maps


_nc_cache = None


def _get_nc():
    global _nc_cache
    if _nc_cache is None:
        _nc_cache = _build()
    return _nc_cache


def kernel(x, cos, sin, Wq, Wkv, Wout, bout, _trace=False):
    x = np.asarray(x, dtype=np.float32)
    cos = np.asarray(cos, dtype=np.float32)
    sin = np.asarray(sin, dtype=np.float32)
    Wq = np.asarray(Wq, dtype=np.float32)
    Wkv = np.asarray(Wkv, dtype=np.float32)
    Wout = np.asarray(Wout, dtype=np.float32)
    bout = np.asarray(bout, dtype=np.float32)

    nc = _get_nc()
    in_maps = _host_inputs(x, cos, sin, Wq, Wkv, Wout)
    res = run_bass_kernel_spmd(nc, in_maps, list(range(8)), trace=_trace)

    y = np.zeros((C, NTOK), np.float32)
    for c in range(8):
        y += res.results[c]["yT"].astype(np.float32)
    out = y.T.reshape(B, NSEQ, C) + bout
    if _trace:
        return out, res
    return out
